# revision 13
# baseline (speedup 1.0000x reference)
"""Trainium2 Bass kernel for DepST_RNN (dependency-tree GNN message passing).

Contract: kernel(**inputs) takes FULL inputs, returns FULL output
[B, N, NODE+DEP] float32.  One NeuronCore per sentence (B=8 data-parallel).

V2: matmul-only dataflow — zero SWDGE ops on the critical path.
All indirection (edge gather, scatter-mean, provenance) is baked on host
into one-hot / scaled selection matrices, so every per-layer step is a PE
matmul:
  * uniform slot layout: WR slots per relation per layer (SW = R*WR),
    per-core slot assignment is data (ctxg / oh / Sp / ohf), the
    instruction stream is identical across cores (SPMD, no envelopes).
  * ctx pass: relation-major Wc matmuls over ctxg [256, L*SW] -> mc.
  * per layer l: gather child ct = sum_p chist_p.T @ oh[p,l] (l matmuls),
    40 relation matmuls Wd[r] @ ct[:, r-slots], DVE evac fused with mc add,
    PE transposes to slot-rows, scatter matmul chout = Sp_l.T @ msgS
    (mask/count scaling baked into Sp), evac to compact bf16 chist_l.
  * final: childT = sum_p chist_p.T @ ohf_p, overlapping layer 7.
"""

import sys

sys.path.insert(0, "/opt/trn_rl_repo")

from contextlib import ExitStack

import numpy as np
import ml_dtypes

import concourse.bass as bass
import concourse.bacc as bacc
import concourse.mybir as mybir
from concourse import tile
from concourse.bass_utils import run_bass_kernel_spmd

B, L, E, N = 8, 8, 128, 1024
NODE, DEP, R = 256, 128, 40

BF16 = mybir.dt.bfloat16
F32 = mybir.dt.float32

NPBF16 = ml_dtypes.bfloat16


def prep(context, dep_W, heads, tails, rels, mask):
    """Host-side structure + per-core input tensors."""
    ctx_np = np.asarray(context, np.float32)
    W_np = np.asarray(dep_W, np.float32)
    heads = np.asarray(heads)
    tails = np.asarray(tails)
    rels = np.asarray(rels)
    mask_np = np.asarray(mask, np.float32)

    # uniform relation-slot width across cores/layers
    cnt = np.zeros((B, L, R), np.int64)
    for b in range(B):
        for l in range(L):
            cnt[b, l] = np.bincount(rels[b, l], minlength=R)
    WR = int(cnt.max())
    SW = R * WR                 # slots per layer
    NT = (SW + 127) // 128      # transpose/scatter chunks (last may be partial)
    GW = L * SW                 # ctxg columns: col = r*(L*WR) + l*WR + j
    NOH = L * (L - 1) // 2      # oh tiles: (p, l) p < l, seq = l(l-1)/2 + p

    st = dict(WR=WR, SW=SW, NT=NT, GW=GW, NOH=NOH)

    # shared weight layouts (relation-chunk-major so DMA chunks pipeline)
    wc_np = np.zeros((128, 2 * R * 128), np.float32)   # (2r + c) blocks
    wd_np = np.zeros((128, R * 128), np.float32)
    for r in range(R):
        for c in range(2):
            wc_np[:, (2 * r + c) * 128:(2 * r + c + 1) * 128] = (
                W_np[r, :, c * 128:(c + 1) * 128].T
            )
        wd_np[:, r * 128:(r + 1) * 128] = W_np[r, :, NODE:].T
    wc_np = wc_np.astype(NPBF16)
    wd_np = wd_np.astype(NPBF16)
    ident_np = np.eye(128, dtype=np.float32)

    in_maps = []
    for b in range(B):
        # slot assignment + provenance + compact row maps
        slot = np.zeros((L, E), np.int64)
        for l in range(L):
            c = np.zeros(R, np.int64)
            for e in np.argsort(rels[b, l], kind="stable"):
                r = int(rels[b, l, e])
                slot[l, e] = r * WR + c[r]
                c[r] += 1
        prov = np.full(N, -1, np.int64)
        provs, uidx = [], []
        for l in range(L):
            provs.append(prov.copy())
            hs = sorted(set(heads[b, l].tolist()))
            assert len(hs) <= 128
            uidx.append({h: i for i, h in enumerate(hs)})
            prov[heads[b, l]] = l
        provF = prov

        ctxg = np.zeros((2 * 128, GW), np.float32)
        ohall = np.zeros((128, NOH * SW), np.float32)
        spt = np.zeros((128, L * NT * 128), np.float32)
        ohf = np.zeros((128, L * N), np.float32)
        for l in range(L):
            cv = np.zeros(N, np.float32)
            np.add.at(cv, heads[b, l], mask_np[b, l])
            for e in range(E):
                s = int(slot[l, e])
                g = int(rels[b, l, e]) * (L * WR) + l * WR + (s % WR)
                t, h = int(tails[b, l, e]), int(heads[b, l, e])
                ctxg[:, g] = ctx_np[b, t, :]
                p = int(provs[l][t])
                if p >= 0:
                    ohall[uidx[p][t], (l * (l - 1) // 2 + p) * SW + s] = 1.0
                # Sp lhsT chunk: [slot % 128 partition, (l*NT + s//128)*128 + u]
                u = uidx[l][h]
                spt[s % 128, (l * NT + s // 128) * 128 + u] = (
                    mask_np[b, l, e] / max(float(cv[h]), 1.0)
                )
        for n in range(N):
            p = int(provF[n])
            if p >= 0:
                ohf[uidx[p][n], p * N + n] = 1.0

        in_maps.append(
            dict(
                ctxg=ctxg.astype(NPBF16),
                wc=wc_np,
                wd=wd_np,
                ohall=ohall.astype(NPBF16),
                spt=spt.astype(NPBF16),
                ohf=ohf.astype(NPBF16),
                ident=ident_np,
            )
        )
    return st, in_maps


def build(nc, st):
    WR, SW, NT, GW, NOH = st["WR"], st["SW"], st["NT"], st["GW"], st["NOH"]
    LWR = L * WR  # ctxg relation-block width

    d_ctxg = nc.declare_dram_parameter("ctxg", [256, GW], BF16, isOutput=False)
    d_wc = nc.declare_dram_parameter("wc", [128, 2 * R * 128], BF16, isOutput=False)
    d_wd = nc.declare_dram_parameter("wd", [128, R * 128], BF16, isOutput=False)
    d_oh = nc.declare_dram_parameter("ohall", [128, NOH * SW], BF16, isOutput=False)
    d_spt = nc.declare_dram_parameter("spt", [128, L * NT * 128], BF16, isOutput=False)
    d_ohf = nc.declare_dram_parameter("ohf", [128, L * N], BF16, isOutput=False)
    d_ident = nc.declare_dram_parameter("ident", [128, 128], F32, isOutput=False)
    d_out = nc.declare_dram_parameter("childT", [128, 1024], BF16, isOutput=True)

    NG = 8                      # relation groups for DMA/compute pipelining
    RG = R // NG                # relations per group
    RA = 13                     # relations in md bank A (slots 0:130)
    SA = RA * WR

    with ExitStack() as ctx:
        tc = ctx.enter_context(tile.TileContext(nc))

        pers = ctx.enter_context(tc.tile_pool(name="pers", bufs=1))

        def sb(name, shape, dt):
            return pers.tile(shape, dt, tag=name, name=name)

        ctxg0 = sb("ctxg0", [128, GW], BF16)
        ctxg1 = sb("ctxg1", [128, GW], BF16)
        wc = sb("wc_sb", [128, 2 * R * 128], BF16)
        wd = sb("wd_sb", [128, R * 128], BF16)
        ohsb = sb("oh_sb", [128, NOH * SW], BF16)
        spt = sb("spt_sb", [128, L * NT * 128], BF16)
        ohfsb = sb("ohf_sb", [128, L * N], BF16)
        ident = sb("ident_sb", [128, 128], F32)
        mcsb = sb("mcsb", [128, GW], BF16)
        chist = sb("chist", [128, L * 128], BF16)
        finT = sb("finT", [128, 1024], BF16)

        pool = ctx.enter_context(tc.tile_pool(name="work", bufs=2))
        pp_wide = ctx.enter_context(tc.tile_pool(name="ps_wide", bufs=2, space="PSUM"))
        pp_ct = ctx.enter_context(tc.tile_pool(name="ps_ct", bufs=1, space="PSUM"))
        pp_mda = ctx.enter_context(tc.tile_pool(name="ps_mda", bufs=1, space="PSUM"))
        pp_mdb = ctx.enter_context(tc.tile_pool(name="ps_mdb", bufs=1, space="PSUM"))
        pp_t = ctx.enter_context(tc.tile_pool(name="ps_t", bufs=2, space="PSUM"))
        pp_ch = ctx.enter_context(tc.tile_pool(name="ps_ch", bufs=1, space="PSUM"))

        # ---- PE warm-up: real matmuls on a zero tile so the HAM clock gate
        # opens before the ctx burst ----
        zsb = sb("zsb", [128, 128], BF16)
        nc.vector.memset(zsb[:, :], 0.0)
        for k in range(48):
            tpw = pp_t.tile([128, 256], F32, tag="tp", name="warm")
            nc.tensor.matmul(
                tpw[:, 0:128], zsb[:, :], zsb[:, :], start=True, stop=True
            )

        # ---- input DMAs: need-time order, alternating across both HWDGE
        # queues (sync + scalar) so queue occupancy isn't the pacer ----
        dmas = []
        for g in range(NG):
            a, w = g * RG * LWR, RG * LWR
            dmas.append((ctxg0[:, a:a + w], d_ctxg[0:128, a:a + w]))
            dmas.append((ctxg1[:, a:a + w], d_ctxg[128:256, a:a + w]))
            aw, ww = g * RG * 2 * 128, RG * 2 * 128
            dmas.append((wc[:, aw:aw + ww], d_wc[:, aw:aw + ww]))
            if g == 1:
                dmas.append((ident[:, :], d_ident[:, :]))
                dmas.append((spt[:, 0:NT * 128], d_spt[:, 0:NT * 128]))
            if g == 3:
                dmas.append((wd[:, 0:20 * 128], d_wd[:, 0:20 * 128]))
            if g == 5:
                dmas.append((wd[:, 20 * 128:R * 128], d_wd[:, 20 * 128:R * 128]))
        for l in range(1, L):
            a = (l * (l - 1) // 2) * SW
            w = l * SW
            dmas.append((ohsb[:, a:a + w], d_oh[:, a:a + w]))
            a = l * NT * 128
            dmas.append((spt[:, a:a + NT * 128], d_spt[:, a:a + NT * 128]))
        dmas.append((ohfsb[:, :], d_ohf[:, :]))
        for i, (dst, src) in enumerate(dmas):
            eng = nc.sync if i % 2 == 0 else nc.scalar
            eng.dma_start(dst, src)

        # ---- ctx pass: relation-major Wc matmuls into rotating PSUM tiles ----
        # psum tile width 512 = 6.4 relation blocks (LWR=80); emit matmuls per
        # (relation, k-chunk) split at tile boundaries.
        NCT = (GW + 511) // 512
        for t in range(NCT):
            t0, t1 = 512 * t, min(512 * (t + 1), GW)
            ps = pp_wide.tile([128, 512], F32, tag="wide", name=f"msgc{t}")
            r_lo, r_hi = t0 // LWR, (t1 - 1) // LWR
            for r in range(r_lo, r_hi + 1):
                a = max(r * LWR, t0)
                bnd = min((r + 1) * LWR, t1)
                if a >= bnd:
                    continue
                for c in (0, 1):
                    src = ctxg0 if c == 0 else ctxg1
                    nc.tensor.matmul(
                        ps[:, a - t0:bnd - t0],
                        wc[:, (2 * r + c) * 128:(2 * r + c + 1) * 128],
                        src[:, a:bnd],
                        start=(c == 0),
                        stop=(c == 1),
                    )
            nc.vector.tensor_copy(mcsb[:, t0:t1], ps[:, 0:t1 - t0])

        # identity in bf16 for the mc-add matmul
        identb = sb("identb", [128, 128], BF16)
        nc.vector.tensor_copy(identb[:, :], ident[:, :])

        # ---- recursion over layers ----
        for l in range(L):
            # mc contribution via identity matmul (off the critical chain:
            # depends only on mcsb).  md split across two PSUM banks so the
            # bank-A evac overlaps the bank-B relation matmuls.
            mcv = mcsb[:, :].rearrange("p (r lw) -> p r lw", lw=LWR)[
                :, :, l * WR:(l + 1) * WR
            ]
            md_a = pp_mda.tile([128, SA], F32, tag="mda", name="mda")
            md_b = pp_mdb.tile([128, SW - SA], F32, tag="mdb", name="mdb")
            nc.tensor.matmul(
                md_a[:, :].rearrange("p (r w) -> p r w", w=WR),
                identb[:, :],
                mcv[:, 0:RA, :],
                start=True,
                stop=(l == 0),
            )
            nc.tensor.matmul(
                md_b[:, :].rearrange("p (r w) -> p r w", w=WR),
                identb[:, :],
                mcv[:, RA:, :],
                start=True,
                stop=(l == 0),
            )
            if l > 0:
                # gather child: ct = sum_p chist_p.T @ oh[p, l]
                ctp = pp_ct.tile([128, SW], F32, tag="ct", name="ct")
                base = (l * (l - 1) // 2) * SW
                for p in range(l):
                    nc.tensor.matmul(
                        ctp[:, :],
                        chist[:, p * 128:(p + 1) * 128],
                        ohsb[:, base + p * SW:base + (p + 1) * SW],
                        start=(p == 0),
                        stop=(p == l - 1),
                    )
                ctsb = pool.tile([128, SW], BF16, tag="ctsb", name="ctsb")
                nc.vector.tensor_copy(ctsb[:, :], ctp[:, :])
                # relation matmuls accumulate onto the mc term
                for r in range(R):
                    md = md_a if r < RA else md_b
                    c0 = r * WR - (0 if r < RA else SA)
                    nc.tensor.matmul(
                        md[:, c0:c0 + WR],
                        wd[:, r * 128:(r + 1) * 128],
                        ctsb[:, r * WR:(r + 1) * WR],
                        start=False,
                        stop=(r == RA - 1 or r == R - 1),
                        skip_group_check=True,
                    )
            # evac to SBUF (bank A early, overlapping bank-B matmuls)
            msum = pool.tile([128, SW], F32, tag="msum", name="msum")
            nc.vector.tensor_copy(msum[:, 0:SA], md_a[:, :])
            nc.vector.tensor_copy(msum[:, SA:], md_b[:, :])
            # transpose to slot-rows (pairs share a PSUM bank) + scatter matmul
            chp = pp_ch.tile([128, 128], F32, tag="chout", name="chout")
            NP = (NT + 1) // 2
            for q in range(NP):
                tp = pp_t.tile([128, 256], F32, tag="tp", name="tp")
                msgS = pool.tile([128, 256], BF16, tag="msgS", name="msgS")
                for h in range(2):
                    t = 2 * q + h
                    if t >= NT:
                        continue
                    c0, c1 = 128 * t, min(128 * (t + 1), SW)
                    cw = c1 - c0
                    nc.tensor.transpose(
                        tp[0:cw, 128 * h:128 * (h + 1)], msum[:, c0:c1], ident[:, :]
                    )
                if q == 0:
                    nc.scalar.copy(msgS[:, :], tp[:, :])
                else:
                    nc.vector.tensor_copy(msgS[:, :], tp[:, :])
                for h in range(2):
                    t = 2 * q + h
                    if t >= NT:
                        continue
                    cw = min(128 * (t + 1), SW) - 128 * t
                    nc.tensor.matmul(
                        chp[:, :],
                        spt[0:cw, (l * NT + t) * 128:(l * NT + t + 1) * 128],
                        msgS[0:cw, 128 * h:128 * (h + 1)],
                        start=(t == 0),
                        stop=(t == NT - 1),
                    )
            nc.scalar.copy(chist[:, l * 128:(l + 1) * 128], chp[:, :])

        # ---- final: childT = sum_p chist_p.T @ ohf_p ----
        fin0 = pp_wide.tile([128, 512], F32, tag="wide", name="fin0")
        fin1 = pp_wide.tile([128, 512], F32, tag="wide", name="fin1")
        for p in range(L):
            for c, ps in enumerate((fin0, fin1)):
                nc.tensor.matmul(
                    ps[:, :],
                    chist[:, p * 128:(p + 1) * 128],
                    ohfsb[:, p * N + c * 512:p * N + (c + 1) * 512],
                    start=(p == 0),
                    stop=(p == L - 1),
                )
        nc.vector.tensor_copy(finT[:, 0:512], fin0[:, :])
        nc.vector.tensor_copy(finT[:, 512:1024], fin1[:, :])
        nc.sync.dma_start(d_out[:, :], finT[:, :])
    return nc


def run(inputs, trace=False, ncores=B, **kw):
    st, in_maps = prep(**inputs)
    nc = bacc.Bacc()
    build(nc, st)
    nc.finalize()
    res = run_bass_kernel_spmd(nc, in_maps[:ncores], list(range(ncores)), trace=trace, **kw)
    ctx_np = np.asarray(inputs["context"], np.float32)
    out = np.zeros((B, N, NODE + DEP), np.float32)
    for b in range(ncores):
        chT = np.asarray(res.results[b]["childT"]).astype(np.float32)
        out[b, :, :NODE] = ctx_np[b]
        out[b, :, NODE:] = chT.T
    return out, res


def kernel(**inputs):
    out, _ = run(inputs)
    return out
